# revision 7
# baseline (speedup 1.0000x reference)
"""MoE SwiGLU block (E=8 experts, top-k=2) on 8 Trainium2 NeuronCores.

Strategy: expert-parallel, one expert per core.
  Host: build comb[t,e] from top_k indices/values, gather each expert's
        tokens (padded to capacity C), pre-transpose/pack to bf16.
  Device (per core, transposed space so weights are used in natural layout):
        G^T = wg^T @ X^T        [I, C]   (lhsT = wg tiles, rhs = X^T)
        U^T = wu^T @ X^T        [I, C]
        A^T = silu(G^T) * U^T   [I, C]   (ACT silu + DVE mul)
        O^T = wd^T @ A^T        [H, C]   (lhsT = wd tiles, rhs = A^T)
  Host: scale by routing weights and scatter-add into the [T, H] output.

All matmuls are bf16 x bf16 -> fp32 PSUM. No on-chip transposes needed.
"""

import os

import numpy as np
import ml_dtypes

E, H, I, K, T = 8, 1024, 2816, 2, 2048
P = 128
KB = H // P   # 8 contraction chunks for the up/gate matmuls
NI = I // P   # 22 I-tiles
NH = H // P   # 8 H-tiles

BF16 = ml_dtypes.bfloat16

_KERNEL_CACHE: dict = {}
_PACK_CACHE: dict = {}
LAST_RESULT = None


def _chunks(total, step):
    out = []
    c0 = 0
    while c0 < total:
        out.append((c0, min(step, total - c0)))
        c0 += step
    return out


def _build_bass(C):
    import concourse.bacc as bacc
    import concourse.mybir as mybir
    from concourse.bass import MemorySpace
    from concourse.tile import TileContext

    dt = mybir.dt
    nc = bacc.Bacc()

    xt = nc.declare_dram_parameter("xt", [P, KB, C], dt.bfloat16, isOutput=False)
    wg = nc.declare_dram_parameter("wg", [NI, P, KB * P], dt.bfloat16, isOutput=False)
    wu = nc.declare_dram_parameter("wu", [NI, P, KB * P], dt.bfloat16, isOutput=False)
    wd = nc.declare_dram_parameter("wd", [NH, P, NI * P], dt.bfloat16, isOutput=False)
    ot = nc.declare_dram_parameter("ot", [NH, P, C], dt.float32, isOutput=True)

    cchunks = _chunks(C, 512)

    with TileContext(nc) as tc:
        with (
            tc.tile_pool(name="xpool", bufs=1) as xpool,
            tc.tile_pool(name="atpool", bufs=1) as atpool,
            tc.tile_pool(name="wpool", bufs=1) as wpool,
            tc.tile_pool(name="wdpool", bufs=1) as wdpool,
            tc.tile_pool(name="spool", bufs=3) as spool,
            tc.tile_pool(name="opool", bufs=2) as opool,
            tc.tile_pool(name="pg", bufs=2, space=MemorySpace.PSUM) as pg_pool,
            tc.tile_pool(name="pu", bufs=2, space=MemorySpace.PSUM) as pu_pool,
            tc.tile_pool(name="po", bufs=2, space=MemorySpace.PSUM) as po_pool,
        ):
            # Resident tiles: input activations and the silu(g)*u intermediate.
            xtile = xpool.tile([P, KB, C], dt.bfloat16)
            nc.sync.dma_start(out=xtile[:], in_=xt[:])
            atile = atpool.tile([P, NI, C], dt.bfloat16)

            # All weights are SBUF-resident (one slot per tile, never
            # recycled): a recycled slot's load-DMA needs 3 sync waits
            # (PE release + 2 DMA queues) but the DMA instruction format
            # only supports 2 — walrus rejects the kernel.
            wgts = []
            wuts = []
            for i in range(NI):
                wgt = wpool.tile([P, KB * P], dt.bfloat16, tag=f"wg{i}")
                wut = wpool.tile([P, KB * P], dt.bfloat16, tag=f"wu{i}")
                nc.sync.dma_start(out=wgt[:], in_=wg[i])
                nc.sync.dma_start(out=wut[:], in_=wu[i])
                wgts.append(wgt)
                wuts.append(wut)
            wdts = []
            for h in range(NH):
                wdt = wdpool.tile([P, NI * P], dt.bfloat16, tag=f"wd{h}")
                nc.sync.dma_start(out=wdt[:], in_=wd[h])
                wdts.append(wdt)

            # Phase 1: A^T[i] = silu(wg_i^T @ X^T) * (wu_i^T @ X^T)
            for i in range(NI):
                wgt, wut = wgts[i], wuts[i]
                for (c0, cs) in cchunks:
                    psg = pg_pool.tile([P, 512], dt.float32, tag="psg")
                    psu = pu_pool.tile([P, 512], dt.float32, tag="psu")
                    for kb in range(KB):
                        nc.tensor.matmul(
                            psg[:, :cs],
                            wgt[:, kb * P : (kb + 1) * P],
                            xtile[:, kb, c0 : c0 + cs],
                            start=(kb == 0),
                            stop=(kb == KB - 1),
                        )
                    for kb in range(KB):
                        nc.tensor.matmul(
                            psu[:, :cs],
                            wut[:, kb * P : (kb + 1) * P],
                            xtile[:, kb, c0 : c0 + cs],
                            start=(kb == 0),
                            stop=(kb == KB - 1),
                        )
                    sg = spool.tile([P, 512], dt.bfloat16, tag="sg")
                    nc.scalar.activation(
                        sg[:, :cs], psg[:, :cs], mybir.ActivationFunctionType.Silu
                    )
                    nc.vector.tensor_mul(
                        atile[:, i, c0 : c0 + cs], sg[:, :cs], psu[:, :cs]
                    )

            # Phase 2: O^T[h] = sum_i wd[i,h]^T @ A^T[i]
            for h in range(NH):
                wdt = wdts[h]
                for (c0, cs) in cchunks:
                    pso = po_pool.tile([P, 512], dt.float32, tag="pso")
                    for i in range(NI):
                        nc.tensor.matmul(
                            pso[:, :cs],
                            wdt[:, i * P : (i + 1) * P],
                            atile[:, i, c0 : c0 + cs],
                            start=(i == 0),
                            stop=(i == NI - 1),
                        )
                    otile = opool.tile([P, 512], dt.float32, tag="otile")
                    nc.vector.tensor_copy(otile[:, :cs], pso[:, :cs])
                    nc.sync.dma_start(out=ot[h, :, c0 : c0 + cs], in_=otile[:, :cs])

    nc.compile()
    return nc


def _get_kernel(C):
    if C not in _KERNEL_CACHE:
        _KERNEL_CACHE[C] = _build_bass(C)
    return _KERNEL_CACHE[C]


def _pack_weights(w_gate, w_up, w_down):
    key = (id(w_gate), id(w_up), id(w_down))
    if key in _PACK_CACHE:
        return _PACK_CACHE[key]
    w_gate = np.asarray(w_gate, dtype=np.float32)
    w_up = np.asarray(w_up, dtype=np.float32)
    w_down = np.asarray(w_down, dtype=np.float32)
    # wg_pack[e, i, p, kb*P+m] = w_gate[e, kb*P+p, i*P+m]
    wg_pack = np.ascontiguousarray(
        w_gate.reshape(E, KB, P, NI, P).transpose(0, 3, 2, 1, 4).reshape(E, NI, P, KB * P)
    ).astype(BF16)
    wu_pack = np.ascontiguousarray(
        w_up.reshape(E, KB, P, NI, P).transpose(0, 3, 2, 1, 4).reshape(E, NI, P, KB * P)
    ).astype(BF16)
    # wd_pack[e, h, p, i*P+m] = w_down[e, i*P+p, h*P+m]
    wd_pack = np.ascontiguousarray(
        w_down.reshape(E, NI, P, NH, P).transpose(0, 3, 2, 1, 4).reshape(E, NH, P, NI * P)
    ).astype(BF16)
    _PACK_CACHE.clear()
    _PACK_CACHE[key] = (wg_pack, wu_pack, wd_pack)
    return _PACK_CACHE[key]


def _setup_trace():
    """Register the NTFF profile hook that sitecustomize's boot() skipped
    (the image's antenv lacks axon_hooks). Dev/profiling only."""
    import sys
    import types

    if "antenv.axon_hooks" not in sys.modules:
        import antenv

        m = types.ModuleType("antenv.axon_hooks")
        m._HOOK = None

        def _set(h):
            m._HOOK = h

        def _get():
            return m._HOOK

        m.set_axon_ntff_profile_hook = _set
        m.get_axon_ntff_profile_hook = _get
        sys.modules["antenv.axon_hooks"] = m
        antenv.axon_hooks = m

    from antenv.axon_hooks import (
        get_axon_ntff_profile_hook,
        set_axon_ntff_profile_hook,
    )

    if get_axon_ntff_profile_hook() is None:
        from trn_agent_boot.trn_boot import _ntff_profile_via_ctypes

        set_axon_ntff_profile_hook(
            _ntff_profile_via_ctypes("/opt/axon/libaxon_pjrt.so")
        )

    import concourse.bass_utils as bu

    bu.upload_artifacts = lambda tmpdir: "local://" + tmpdir


def kernel(hidden_states, top_k_indices, top_k_values, w_gate, w_up, w_down):
    global LAST_RESULT
    from concourse.bass_utils import run_bass_kernel_spmd

    hs = np.asarray(hidden_states, dtype=np.float32)
    idx = np.asarray(top_k_indices)
    val = np.asarray(top_k_values, dtype=np.float32)

    # comb[t, e] = sum_k val[t, k] * (idx[t, k] == e)
    comb = np.zeros((T, E), np.float32)
    np.add.at(
        comb,
        (np.repeat(np.arange(T), idx.shape[1]), idx.ravel()),
        val.ravel(),
    )

    tok_lists = [np.nonzero(comb[:, e])[0] for e in range(E)]
    counts = [len(tk) for tk in tok_lists]
    C = max(128, ((max(counts) + P - 1) // P) * P)

    wg_pack, wu_pack, wd_pack = _pack_weights(w_gate, w_up, w_down)

    in_maps = []
    for e in range(E):
        tk = tok_lists[e]
        xp = np.zeros((C, H), np.float32)
        xp[: counts[e]] = hs[tk]
        # xt_pack[p, kb, c] = X[c, kb*P+p]
        xt_pack = np.ascontiguousarray(
            xp.reshape(C, KB, P).transpose(2, 1, 0)
        ).astype(BF16)
        in_maps.append(
            {
                "xt": xt_pack,
                "wg": wg_pack[e],
                "wu": wu_pack[e],
                "wd": wd_pack[e],
            }
        )

    nc = _get_kernel(C)
    trace = os.environ.get("MOE_TRACE", "0") == "1"
    kwargs = {}
    if trace:
        _setup_trace()
        tracedir = os.environ.get("MOE_TRACE_DIR")
        if tracedir:
            os.makedirs(tracedir, exist_ok=True)
            kwargs["tmpdir"] = tracedir
    res = run_bass_kernel_spmd(nc, in_maps, list(range(E)), trace=trace, **kwargs)
    LAST_RESULT = res

    out = np.zeros((T, H), np.float32)
    for e in range(E):
        tk = tok_lists[e]
        n = counts[e]
        # ot [NH, P, C] -> y [C, H]
        y = res.results[e]["ot"].transpose(2, 0, 1).reshape(C, H)[:n]
        out[tk] += y * comb[tk, e][:, None]
    return out
